# revision 35
# baseline (speedup 1.0000x reference)
"""Fused multi-head attention (B=2, N=2048, C=1024, H=16) on 8 TRN2 NeuronCores.

Sharding: core = (b, g) with b = batch (2) and g = head-group of 4 heads (4).
Each core computes, for its batch and 4 heads:
    qkv slice -> per-head softmax attention -> out-proj partial (row-parallel).
Host sums the 4 per-head-group proj partials per batch and adds b_proj.

Device algorithm (per core), matmuls in bf16:
  qkT/kT = (x @ Wqk)^T   [q/k feats on partitions, 2048 tokens]
  v      = x @ Wv        [2048 tokens, 4*64] (+ ones column per head)
  attention, software-pipelined over 64 global (block, kc2) steps:
    S^T tiles = matmul(lhsT=kTp_h (zero-padded K=128), rhs=q-chunk)
    expST = exp(S^T/8)  (ScalarE, PSUM->SBUF)
    outT[65, rows] += [v_h|1]^T-matmul expST  (row 64 = softmax denom)
  The ST matmuls for step g+1 are emitted BEFORE the PV matmuls of step g
  so the PE (strictly in-order) always has the next exp's input ready while
  ScalarE drains the current exps: steady state is ScalarE-bound at
  ~2.2us/step.  qk/v/proj fill work is interleaved into per-step slack.
  outT[0:64] *= 1/denominator  (fast DVE recip, GpSimd bcast, DVE mult)
  proj partials = out^T-matmul Wp -> DMA out
"""

import os

import numpy as np

import concourse.bass as bass
import concourse.mybir as mybir
import concourse.tile as tile
from concourse import bacc
from concourse.bass_utils import run_bass_kernel_spmd

B, N, C = 2, 2048, 1024
HC = 4  # heads per core
D = 64
NCORES = 8
KC = C // 128  # 8 contraction chunks for qkv
SCALE = D**-0.5  # 0.125

MM_DT = os.environ.get("ATTN_MM_DT", "bf16")


def _np_in_dtype():
    if MM_DT == "bf16":
        import ml_dtypes

        return np.dtype(ml_dtypes.bfloat16)
    return np.dtype(np.float32)


def _prep(a):
    """Cast to the device input dtype; for f32r, pre-round to TF32 (RTNE)."""
    a = np.ascontiguousarray(a)
    if MM_DT != "f32r":
        return a.astype(_np_in_dtype())
    u = a.astype(np.float32).view(np.uint32)
    u = (u + 0x0FFF + ((u >> 13) & 1)) & np.uint32(0xFFFFE000)
    return u.view(np.float32)


def build_nc():
    f32 = mybir.dt.float32
    in_dt = {
        "bf16": mybir.dt.bfloat16,
        "f32r": mybir.dt.float32r,
        "f32": mybir.dt.float32,
    }[MM_DT]

    nc = bacc.Bacc("TRN2", target_bir_lowering=False, debug=False, num_devices=NCORES)
    xT_d = nc.dram_tensor("xT", [C, N], in_dt, kind="ExternalInput").ap()
    wqk_d = nc.dram_tensor("wqk", [C, 2 * HC * D], in_dt, kind="ExternalInput").ap()
    wv_d = nc.dram_tensor("wv", [C, HC * D], in_dt, kind="ExternalInput").ap()
    wp_d = nc.dram_tensor("wp", [HC * D, C], in_dt, kind="ExternalInput").ap()
    # proj partials leave in bf16 (hosts sums in f32): halves out-DMA bytes;
    # adds ~1.7e-3 rel err (measured), well inside the margin
    out_d = nc.dram_tensor("out", [N, C], in_dt, kind="ExternalOutput").ap()

    with tile.TileContext(nc) as tc:
        with (
            tc.tile_pool(name="const", bufs=1) as const,
            tc.tile_pool(name="ex", bufs=8) as expool,
            tc.tile_pool(name="den", bufs=6) as dpool,
            tc.tile_pool(name="stage", bufs=4) as stage,
            tc.tile_pool(name="stps", bufs=2, space="PSUM") as stps,
            tc.tile_pool(name="pvps", bufs=2, space="PSUM") as pvps,
            tc.tile_pool(name="pvhold", bufs=2, space="PSUM") as pvhold,
        ):
            # persistent tiles
            # qkT/kT chunks: idx 0 = heads 0,1; idx 1 = heads 2,3
            #   (head even -> partitions 0:64, odd -> 64:128)
            qkT_sb = const.tile([128, 2, N], in_dt, tag="qkT")
            # kTp: per-head zero-padded K=128 stationary operand: head even
            #   has kT in rows 0:64 / zeros in 64:128, head odd the reverse,
            #   so full-array matmuls select one head's contraction.
            #   (64-row half-array ST matmuls measure wrong on HW when
            #   interleaved with full-array PV loads.)
            kTp_sb = const.tile([128, HC, N], in_dt, tag="kTp")
            v_sb = const.tile([128, 16, HC, D + 1], in_dt, tag="v")
            wp_sb = const.tile([128, 2, C], in_dt, tag="wp")
            outT_sb = const.tile([128, 2, N], in_dt, tag="outT")
            xT_sb = const.tile([128, KC, N], in_dt, tag="xT")
            wqk_sb = const.tile([128, KC, 2 * HC * D], in_dt, tag="wqk")
            wv_sb = const.tile([128, KC, HC * D], in_dt, tag="wv")

            # PE p-state warmers: dummy matmuls on a zeroed tile keep the PE
            # ramping to 2.4GHz during the DMA wait so the first real qk
            # psums don't run at the 0.65GHz cold clock
            zq = const.tile([128, 512], in_dt, tag="zq")
            nc.vector.memset(zq[:], 0.0)
            warm_ps = stps.tile([128, 1024], f32, tag="st", name="st")
            for i in range(12):
                nc.tensor.matmul(
                    warm_ps[:, :512], zq[:, :128], zq[:], start=True, stop=True
                )
            # warm the ScalarE Exp table during the DMA wait so the first
            # real activation doesn't pay the 1.3us table load
            wt_in = const.tile([1, 8], f32, tag="wtin")
            wt_out = const.tile([1, 8], f32, tag="wtout")
            nc.vector.memset(wt_in[:], 0.0)
            # zero the whole kTp on the (otherwise idle) GpSimd during the
            # DMA wait; the k copies then only fill their 64-row halves
            nc.gpsimd.memset(kTp_sb[:], 0.0)

            # ---- DMAs: batched issues split across the two HWDGE queues
            # (SP + ACT).  Host packs wqk columns as [q01|k01|q23|k23], so
            # the first-exp critical set is wqk[:, :256] + xT tokens 0:512.
            nc.sync.dma_start(
                wqk_sb[:, :, 0:256],
                wqk_d[:, 0:256].rearrange("(kc p) c -> p kc c", p=128),
            )
            nc.scalar.dma_start(
                xT_sb[:, :, 0:512],
                xT_d[:, 0:512].rearrange("(kc p) n -> p kc n", p=128),
            )
            nc.scalar.activation(
                wt_out, wt_in, mybir.ActivationFunctionType.Exp, scale=1.0
            )
            nc.sync.dma_start(wv_sb[:], wv_d.rearrange("(kc p) c -> p kc c", p=128))
            nc.sync.dma_start(
                wqk_sb[:, :, 256:512],
                wqk_d[:, 256:512].rearrange("(kc p) c -> p kc c", p=128),
            )
            for nt in range(1, 4):
                nc.scalar.dma_start(
                    xT_sb[:, :, nt * 512 : (nt + 1) * 512],
                    xT_d[:, nt * 512 : (nt + 1) * 512].rearrange(
                        "(kc p) n -> p kc n", p=128
                    ),
                )
            nc.sync.dma_start(wp_sb[:], wp_d.rearrange("(c2 p) c -> p c2 c", p=128))

            # ones column for the softmax-denominator trick
            ones_f32 = const.tile([128, 16, HC, 1], f32, tag="ones")
            nc.vector.memset(ones_f32[:], 1.0)
            nc.vector.tensor_copy(v_sb[:, :, :, D : D + 1], ones_f32[:])

            # ---- emission helpers ----
            def qk_chunk(mf, nt, big=False):
                """One psum of (x @ Wqk)^T: feat chunk mf, token chunk nt.
                wqk feat chunks (host order): 0 = q heads 0,1; 1 = k heads
                0,1; 2 = q heads 2,3; 3 = k heads 2,3."""
                if big:
                    ps = stps.tile([128, 1024], f32, tag="st", name="st")[:, :512]
                else:
                    ps = pvps.tile([128, 512], f32, tag="pv", name="pv")
                for kc in range(KC):
                    nc.tensor.matmul(
                        ps,
                        wqk_sb[:, kc, mf * 128 : (mf + 1) * 128],
                        xT_sb[:, kc, nt * 512 : (nt + 1) * 512],
                        start=(kc == 0),
                        stop=(kc == KC - 1),
                    )
                nts = slice(nt * 512, (nt + 1) * 512)
                if mf % 2 == 0:
                    nc.vector.tensor_copy(qkT_sb[:, mf // 2, nts], ps)
                else:
                    h0, h1 = 2 * (mf // 2), 2 * (mf // 2) + 1
                    nc.vector.tensor_copy(kTp_sb[0:64, h0, nts], ps[0:64, :])
                    nc.vector.tensor_copy(kTp_sb[64:128, h1, nts], ps[64:128, :])

            def v_chunk(t):
                """One psum of v = x @ Wv for token(=key) chunk t, all heads."""
                ps = pvps.tile([128, 512], f32, tag="pv", name="pv")[:, : HC * D]
                for kc in range(KC):
                    nc.tensor.matmul(
                        ps,
                        xT_sb[:, kc, t * 128 : (t + 1) * 128],
                        wv_sb[:, kc, :],
                        start=(kc == 0),
                        stop=(kc == KC - 1),
                    )
                nc.vector.tensor_copy(
                    v_sb[:, t, :, 0:D], ps.rearrange("p (h d) -> p h d", h=HC)
                )

            def proj_pair(t, eng=None):
                """partial[t*128:(t+1)*128, :] = out @ Wp, both column
                halves staged into one wide tile -> one 2KB-row DMA."""
                sg = stage.tile([128, 1024], in_dt, tag="sg", name="sg")
                for nf in range(2):
                    ps = pvps.tile([128, 512], f32, tag="pv", name="pv")
                    for c2 in range(2):
                        nc.tensor.matmul(
                            ps,
                            outT_sb[:, c2, t * 128 : (t + 1) * 128],
                            wp_sb[:, c2, nf * 512 : (nf + 1) * 512],
                            start=(c2 == 0),
                            stop=(c2 == 1),
                        )
                    nc.vector.tensor_copy(sg[:, nf * 512 : (nf + 1) * 512], ps)
                (eng or nc.sync).dma_start(out_d[t * 128 : (t + 1) * 128, :], sg)

            # ---- flat software-pipelined attention schedule ----
            # rc pairs complete early so only proj(3) remains in the tail
            blocks = [(0, 0), (0, 1), (1, 0), (1, 1), (0, 2), (1, 2), (0, 3), (1, 3)]
            seq = [(hp, rc, kc2) for hp, rc in blocks for kc2 in range(8)]

            stp_pend = {}
            ex_pend = {}
            pv_cur = {}

            def emit_st(g):
                hp, rc, kc2 = seq[g]
                stp = {}
                for hi, h in enumerate((2 * hp, 2 * hp + 1)):
                    if kc2 == 0 and g > 0 and hi == 0:
                        # block boundary: borrow two fill-pool banks for
                        # head 0 so its S^T doesn't wait on the stps slot
                        # still being read by the previous block's last exp
                        halves = []
                        for j in range(2):
                            t = pvps.tile([128, 512], f32, tag="pv", name="pv")
                            nc.tensor.matmul(
                                t,
                                kTp_sb[:, h, j * 128 : (j + 1) * 128],
                                qkT_sb[:, hp, rc * 512 : (rc + 1) * 512],
                                start=True,
                                stop=True,
                            )
                            halves.append(t)
                        stp[h] = tuple(halves)
                        continue
                    t = stps.tile([128, 1024], f32, tag="st", name="st")
                    for j in range(2):
                        kc = 2 * kc2 + j
                        nc.tensor.matmul(
                            t[:, j * 512 : (j + 1) * 512],
                            kTp_sb[:, h, kc * 128 : (kc + 1) * 128],
                            qkT_sb[:, hp, rc * 512 : (rc + 1) * 512],
                            start=True,
                            stop=True,
                        )
                    stp[h] = t
                stp_pend[g] = stp

            def emit_exp(g):
                hp, rc, kc2 = seq[g]
                stp = stp_pend.pop(g)
                ex2 = {}
                for h in (2 * hp, 2 * hp + 1):
                    ex = expool.tile([128, 1024], in_dt, tag="ex", name="ex")
                    if isinstance(stp[h], tuple):
                        for j in range(2):
                            nc.scalar.activation(
                                ex[:, j * 512 : (j + 1) * 512],
                                stp[h][j],
                                mybir.ActivationFunctionType.Exp,
                                scale=SCALE,
                            )
                    else:
                        nc.scalar.activation(
                            ex, stp[h], mybir.ActivationFunctionType.Exp, scale=SCALE
                        )
                    ex2[h] = ex
                ex_pend[g] = ex2

            def emit_pv(g):
                hp, rc, kc2 = seq[g]
                heads = (2 * hp, 2 * hp + 1)
                if kc2 == 0:
                    pv_cur[(hp, rc)] = {
                        h: pvhold.tile([128, 512], f32, tag="pvh", name="pvh")
                        for h in heads
                    }
                ex2 = ex_pend.pop(g)
                for h in heads:
                    for j in range(2):
                        kc = 2 * kc2 + j
                        nc.tensor.matmul(
                            pv_cur[(hp, rc)][h][: D + 1, :],
                            v_sb[:, kc, h, :],
                            ex2[h][:, j * 512 : (j + 1) * 512],
                            start=(kc == 0),
                            stop=(kc == 15),
                        )

            def emit_div(hp, rc):
                """Normalize: outT[0:64] = pv[0:64] / pv[64] per head.
                The custom-DVE fast reciprocal silently ignores input
                partition offsets, so first copy the denominator row from
                partition 64 down to a partition-0 tile."""
                heads = (2 * hp, 2 * hp + 1)
                pv = pv_cur.pop((hp, rc))
                dens, rbcs = {}, {}
                for h in heads:
                    dcp = dpool.tile([1, 512], f32, tag="dcp", name="dcp")
                    nc.vector.tensor_copy(dcp, pv[h][D : D + 1, :])
                    dens[h] = dpool.tile([1, 512], f32, tag="den", name="den")
                    nc.vector.reciprocal_approx_fast(dens[h], dcp)
                for h in heads:
                    rbcs[h] = dpool.tile([64, 512], f32, tag="rbc", name="rbc")
                    nc.gpsimd.partition_broadcast(rbcs[h], dens[h])
                for h in heads:
                    hb = (h % 2) * 64
                    nc.vector.tensor_tensor(
                        out=outT_sb[hb : hb + 64, hp, rc * 512 : (rc + 1) * 512],
                        in0=pv[h][0:D, :],
                        in1=rbcs[h][:],
                        op=mybir.AluOpType.mult,
                    )

            # fill work (qk/v/proj) interleaved into the attention steps'
            # PE slack.  g -> list of closures popped after that step's
            # lookahead STs and before its PVs.
            fill_plan = {g: [] for g in range(64)}
            fill_plan[0] = [
                lambda: v_chunk(0),
                lambda: v_chunk(1),
                lambda: qk_chunk(1, 1),
            ]
            fill_plan[1] = [
                lambda: v_chunk(2),
                lambda: v_chunk(3),
                lambda: qk_chunk(1, 2),
            ]
            fill_plan[2] = [
                lambda: v_chunk(4),
                lambda: v_chunk(5),
                lambda: qk_chunk(1, 3),
            ]
            fill_plan[3] = [
                lambda: v_chunk(6),
                lambda: v_chunk(7),
                lambda: qk_chunk(0, 1),
            ]
            fill_plan[4] = [lambda: v_chunk(8), lambda: v_chunk(9)]
            fill_plan[5] = [
                lambda: v_chunk(10),
                lambda: v_chunk(11),
                lambda: qk_chunk(2, 0),
            ]
            fill_plan[6] = [lambda: v_chunk(12), lambda: v_chunk(13)]
            fill_plan[7] = [lambda: v_chunk(14), lambda: v_chunk(15)]
            fill_plan[8] = [lambda: qk_chunk(3, 0)]
            fill_plan[9] = [lambda: qk_chunk(3, 1)]
            fill_plan[10] = [lambda: qk_chunk(3, 2)]
            fill_plan[11] = [lambda: qk_chunk(3, 3)]
            fill_plan[12] = [lambda: qk_chunk(0, 2)]
            fill_plan[14] = [lambda: qk_chunk(2, 1)]
            fill_plan[20] = [lambda: qk_chunk(0, 3)]
            fill_plan[22] = [lambda: qk_chunk(2, 2)]
            fill_plan[33] = [lambda: qk_chunk(2, 3)]
            # proj(rc) may only be emitted after BOTH (0,rc) and (1,rc)
            # divs are emitted (PE is in-order: an early proj MM waiting on
            # outT would deadlock behind itself).
            for i, g in enumerate(range(24, 32, 2)):  # proj(0) after (1,0)@g23
                fill_plan[g].append(lambda t=i: proj_pair(t))
            for i, g in enumerate(range(32, 40, 2)):  # proj(1) after (1,1)@g31
                fill_plan[g].append(lambda t=4 + i: proj_pair(t))
            for i, g in enumerate(range(48, 56, 2)):  # proj(2) after (1,2)@g47
                fill_plan[g].append(lambda t=8 + i: proj_pair(t))

            # prelude: minimum to start step g0 (q01 tokens 0:512 and k01
            # keys 0:512; v chunks follow as the first fills)
            qk_chunk(0, 0, big=True)
            qk_chunk(1, 0, big=True)

            emit_st(0)
            for g in range(64):
                hp, rc, kc2 = seq[g]
                emit_exp(g)
                if g + 1 < 64:
                    emit_st(g + 1)
                for fill in fill_plan[g]:
                    fill()
                emit_pv(g)
                if kc2 == 7:
                    emit_div(hp, rc)
                    if g == 63:
                        # keep the PE p-state high through the final div
                        # chain so the tail proj matmuls run at full clock
                        wps = pvps.tile([128, 512], f32, tag="pv", name="pv")
                        for _ in range(10):
                            nc.tensor.matmul(
                                wps, zq[:, :128], zq[:], start=True, stop=True
                            )

            # tail: proj(3) — all four token chunks staged into one wide
            # tile and shipped as a single DMA (each DMA to HBM pays a ~2us
            # write-receipt latency; one issue instead of four)
            tail_sg = const.tile([128, 4, C], in_dt, tag="tailsg")
            for t in range(12, 16):
                for nf in range(2):
                    ps = pvps.tile([128, 512], f32, tag="pv", name="pv")
                    for c2 in range(2):
                        nc.tensor.matmul(
                            ps,
                            outT_sb[:, c2, t * 128 : (t + 1) * 128],
                            wp_sb[:, c2, nf * 512 : (nf + 1) * 512],
                            start=(c2 == 0),
                            stop=(c2 == 1),
                        )
                    nc.vector.tensor_copy(
                        tail_sg[:, t - 12, nf * 512 : (nf + 1) * 512], ps
                    )
            nc.sync.dma_start(
                out_d[1536:2048, :].rearrange("(t p) c -> p t c", p=128), tail_sg[:]
            )
    nc.compile()
    return nc


def make_in_maps(x, w_qkv, w_proj):
    in_maps = []
    for core in range(NCORES):
        b, g = core // 4, core % 4
        qs = slice(g * 256, (g + 1) * 256)
        q0 = g * 256
        in_maps.append(
            {
                "xT": _prep(x[b].T),
                # column order [q01 | k01 | q23 | k23] so the first-exp
                # critical DMA is just the first 256 columns
                "wqk": _prep(
                    np.concatenate(
                        [
                            w_qkv[:, q0 : q0 + 128],
                            w_qkv[:, C + q0 : C + q0 + 128],
                            w_qkv[:, q0 + 128 : q0 + 256],
                            w_qkv[:, C + q0 + 128 : C + q0 + 256],
                        ],
                        axis=1,
                    )
                ),
                "wv": _prep(w_qkv[:, 2 * C + g * 256 : 2 * C + (g + 1) * 256]),
                "wp": _prep(w_proj[qs, :]),
            }
        )
    return in_maps


def run_hw(x, w_qkv, w_proj, b_proj, trace=False):
    """Returns (full output [2, 2048, 1024] f32, exec_time_ns or None)."""
    in_maps = make_in_maps(x, w_qkv, w_proj)
    nc = build_nc()
    r = run_bass_kernel_spmd(nc, in_maps, core_ids=list(range(NCORES)), trace=trace)
    full = np.zeros((B, N, C), np.float32)
    for core in range(NCORES):
        full[core // 4] += np.asarray(r.results[core]["out"]).astype(np.float32)
    full += np.asarray(b_proj, np.float32)[None, None, :]
    return full, r.exec_time_ns


def kernel(**inputs):
    x = np.asarray(inputs["x"], np.float32)
    w_qkv = np.asarray(inputs["w_qkv"], np.float32)
    w_proj = np.asarray(inputs["w_proj"], np.float32)
    b_proj = np.asarray(inputs["b_proj"], np.float32)
    out, _ = run_hw(x, w_qkv, w_proj, b_proj, trace=False)
    return out


# revision 38
# speedup vs baseline: 1.2003x; 1.2003x over previous
"""Fused multi-head attention (B=2, N=2048, C=1024, H=16) on 8 TRN2 NeuronCores.

Sharding: core = (b, g) with b = batch (2) and g = head-group of 4 heads (4).
Each core computes, for its batch and 4 heads:
    qkv slice -> per-head softmax attention -> out-proj partial (row-parallel).
Host sums the 4 per-head-group proj partials per batch and adds b_proj.

Device algorithm (per core), matmuls in bf16:
  qkT/kT = (x @ Wqk)^T   [q/k feats on partitions, 2048 tokens]
  v      = x @ Wv        [2048 tokens, 4*64] (+ ones column per head)
  attention, software-pipelined over 64 global (block, kc2) steps:
    S^T tiles = matmul(lhsT=kTp_h (zero-padded K=128), rhs=q-chunk)
    expST = exp(S^T/8)  (ScalarE, PSUM->SBUF)
    outT[65, rows] += [v_h|1]^T-matmul expST  (row 64 = softmax denom)
  The ST matmuls for step g+1 are emitted BEFORE the PV matmuls of step g
  so the PE (strictly in-order) always has the next exp's input ready while
  ScalarE drains the current exps: steady state is ScalarE-bound at
  ~2.2us/step.  qk/v/proj fill work is interleaved into per-step slack.
  outT[0:64] *= 1/denominator  (fast DVE recip, GpSimd bcast, DVE mult)
  proj partials = out^T-matmul Wp -> DMA out
"""

import os

import numpy as np

import concourse.bass as bass
import concourse.mybir as mybir
import concourse.tile as tile
from concourse import bacc
from concourse.bass_utils import run_bass_kernel_spmd

B, N, C = 2, 2048, 1024
HC = 4  # heads per core
D = 64
NCORES = 8
KC = C // 128  # 8 contraction chunks for qkv
SCALE = D**-0.5  # 0.125

MM_DT = os.environ.get("ATTN_MM_DT", "bf16")


def _np_in_dtype():
    if MM_DT == "bf16":
        import ml_dtypes

        return np.dtype(ml_dtypes.bfloat16)
    return np.dtype(np.float32)


def _prep(a):
    """Cast to the device input dtype; for f32r, pre-round to TF32 (RTNE)."""
    a = np.ascontiguousarray(a)
    if MM_DT != "f32r":
        return a.astype(_np_in_dtype())
    u = a.astype(np.float32).view(np.uint32)
    u = (u + 0x0FFF + ((u >> 13) & 1)) & np.uint32(0xFFFFE000)
    return u.view(np.float32)


def build_nc():
    f32 = mybir.dt.float32
    in_dt = {
        "bf16": mybir.dt.bfloat16,
        "f32r": mybir.dt.float32r,
        "f32": mybir.dt.float32,
    }[MM_DT]

    nc = bacc.Bacc("TRN2", target_bir_lowering=False, debug=False, num_devices=NCORES)
    xT_d = nc.dram_tensor("xT", [C, N], in_dt, kind="ExternalInput").ap()
    wqk_d = nc.dram_tensor("wqk", [C, 2 * HC * D], in_dt, kind="ExternalInput").ap()
    wv_d = nc.dram_tensor("wv", [C, HC * D], in_dt, kind="ExternalInput").ap()
    wp_d = nc.dram_tensor("wp", [HC * D, C], in_dt, kind="ExternalInput").ap()
    # proj partials leave in bf16 (hosts sums in f32): halves out-DMA bytes;
    # adds ~1.7e-3 rel err (measured), well inside the margin
    out_d = nc.dram_tensor("out", [N, C], in_dt, kind="ExternalOutput").ap()

    with tile.TileContext(nc) as tc:
        with (
            tc.tile_pool(name="const", bufs=1) as const,
            tc.tile_pool(name="ex", bufs=8) as expool,
            tc.tile_pool(name="den", bufs=6) as dpool,
            tc.tile_pool(name="stage", bufs=4) as stage,
            tc.tile_pool(name="stps", bufs=2, space="PSUM") as stps,
            tc.tile_pool(name="pvps", bufs=2, space="PSUM") as pvps,
            tc.tile_pool(name="pvhold", bufs=2, space="PSUM") as pvhold,
        ):
            # persistent tiles
            # qkT/kT chunks: idx 0 = heads 0,1; idx 1 = heads 2,3
            #   (head even -> partitions 0:64, odd -> 64:128)
            qkT_sb = const.tile([128, 2, N], in_dt, tag="qkT")
            # kTp: per-head zero-padded K=128 stationary operand: head even
            #   has kT in rows 0:64 / zeros in 64:128, head odd the reverse,
            #   so full-array matmuls select one head's contraction.
            #   (64-row half-array ST matmuls measure wrong on HW when
            #   interleaved with full-array PV loads.)
            kTp_sb = const.tile([128, HC, N], in_dt, tag="kTp")
            v_sb = const.tile([128, 16, HC, D + 1], in_dt, tag="v")
            wp_sb = const.tile([128, 2, C], in_dt, tag="wp")
            outT_sb = const.tile([128, 2, N], in_dt, tag="outT")
            xT_sb = const.tile([128, KC, N], in_dt, tag="xT")
            wqk_sb = const.tile([128, KC, 2 * HC * D], in_dt, tag="wqk")
            wv_sb = const.tile([128, KC, HC * D], in_dt, tag="wv")

            # PE p-state warmers: dummy matmuls on a zeroed tile keep the PE
            # ramping to 2.4GHz during the DMA wait so the first real qk
            # psums don't run at the 0.65GHz cold clock
            zq = const.tile([128, 512], in_dt, tag="zq")
            nc.vector.memset(zq[:], 0.0)
            warm_ps = stps.tile([128, 1024], f32, tag="st", name="st")
            for i in range(12):
                nc.tensor.matmul(
                    warm_ps[:, :512], zq[:, :128], zq[:], start=True, stop=True
                )
            # warm the ScalarE Exp table during the DMA wait so the first
            # real activation doesn't pay the 1.3us table load
            wt_in = const.tile([1, 8], f32, tag="wtin")
            wt_out = const.tile([1, 8], f32, tag="wtout")
            nc.vector.memset(wt_in[:], 0.0)
            # zero the whole kTp on the (otherwise idle) GpSimd during the
            # DMA wait; the k copies then only fill their 64-row halves
            nc.gpsimd.memset(kTp_sb[:], 0.0)

            # ---- DMAs: batched issues split across the two HWDGE queues
            # (SP + ACT).  Host packs wqk columns as [q01|k01|q23|k23], so
            # the first-exp critical set is wqk[:, :256] + xT tokens 0:512.
            nc.sync.dma_start(
                wqk_sb[:, :, 0:256],
                wqk_d[:, 0:256].rearrange("(kc p) c -> p kc c", p=128),
            )
            nc.scalar.dma_start(
                xT_sb[:, :, 0:512],
                xT_d[:, 0:512].rearrange("(kc p) n -> p kc n", p=128),
            )
            nc.scalar.activation(
                wt_out, wt_in, mybir.ActivationFunctionType.Exp, scale=1.0
            )
            nc.sync.dma_start(wv_sb[:], wv_d.rearrange("(kc p) c -> p kc c", p=128))
            nc.sync.dma_start(
                wqk_sb[:, :, 256:512],
                wqk_d[:, 256:512].rearrange("(kc p) c -> p kc c", p=128),
            )
            for nt in range(1, 4):
                nc.scalar.dma_start(
                    xT_sb[:, :, nt * 512 : (nt + 1) * 512],
                    xT_d[:, nt * 512 : (nt + 1) * 512].rearrange(
                        "(kc p) n -> p kc n", p=128
                    ),
                )
            nc.sync.dma_start(wp_sb[:], wp_d.rearrange("(c2 p) c -> p c2 c", p=128))

            # ones column for the softmax-denominator trick
            ones_f32 = const.tile([128, 16, HC, 1], f32, tag="ones")
            nc.vector.memset(ones_f32[:], 1.0)
            nc.vector.tensor_copy(v_sb[:, :, :, D : D + 1], ones_f32[:])

            # ---- emission helpers ----
            def qk_chunk(mf, nt, big=False):
                """One psum of (x @ Wqk)^T: feat chunk mf, token chunk nt.
                wqk feat chunks (host order): 0 = q heads 0,1; 1 = k heads
                0,1; 2 = q heads 2,3; 3 = k heads 2,3."""
                if big:
                    ps = stps.tile([128, 1024], f32, tag="st", name="st")[:, :512]
                else:
                    ps = pvps.tile([128, 512], f32, tag="pv", name="pv")
                for kc in range(KC):
                    nc.tensor.matmul(
                        ps,
                        wqk_sb[:, kc, mf * 128 : (mf + 1) * 128],
                        xT_sb[:, kc, nt * 512 : (nt + 1) * 512],
                        start=(kc == 0),
                        stop=(kc == KC - 1),
                    )
                nts = slice(nt * 512, (nt + 1) * 512)
                if mf % 2 == 0:
                    nc.vector.tensor_copy(qkT_sb[:, mf // 2, nts], ps)
                else:
                    h0, h1 = 2 * (mf // 2), 2 * (mf // 2) + 1
                    nc.vector.tensor_copy(kTp_sb[0:64, h0, nts], ps[0:64, :])
                    nc.vector.tensor_copy(kTp_sb[64:128, h1, nts], ps[64:128, :])

            def v_chunk(t):
                """One psum of v = x @ Wv for token(=key) chunk t, all heads."""
                ps = pvps.tile([128, 512], f32, tag="pv", name="pv")[:, : HC * D]
                for kc in range(KC):
                    nc.tensor.matmul(
                        ps,
                        xT_sb[:, kc, t * 128 : (t + 1) * 128],
                        wv_sb[:, kc, :],
                        start=(kc == 0),
                        stop=(kc == KC - 1),
                    )
                nc.vector.tensor_copy(
                    v_sb[:, t, :, 0:D], ps.rearrange("p (h d) -> p h d", h=HC)
                )

            def proj_pair(t, eng=None):
                """partial[t*128:(t+1)*128, :] = out @ Wp, both column
                halves staged into one wide tile -> one 2KB-row DMA."""
                sg = stage.tile([128, 1024], in_dt, tag="sg", name="sg")
                for nf in range(2):
                    ps = pvps.tile([128, 512], f32, tag="pv", name="pv")
                    for c2 in range(2):
                        nc.tensor.matmul(
                            ps,
                            outT_sb[:, c2, t * 128 : (t + 1) * 128],
                            wp_sb[:, c2, nf * 512 : (nf + 1) * 512],
                            start=(c2 == 0),
                            stop=(c2 == 1),
                        )
                    nc.vector.tensor_copy(sg[:, nf * 512 : (nf + 1) * 512], ps)
                (eng or nc.sync).dma_start(out_d[t * 128 : (t + 1) * 128, :], sg)

            # ---- flat software-pipelined attention schedule ----
            # rc pairs complete early so only proj(3) remains in the tail
            blocks = [(0, 0), (0, 1), (1, 0), (1, 1), (0, 2), (1, 2), (0, 3), (1, 3)]
            seq = [(hp, rc, kc2) for hp, rc in blocks for kc2 in range(8)]

            stp_pend = {}
            ex_pend = {}
            pv_cur = {}

            def emit_st(g):
                hp, rc, kc2 = seq[g]
                stp = {}
                for h in (2 * hp, 2 * hp + 1):
                    t = stps.tile([128, 1024], f32, tag="st", name="st")
                    for j in range(2):
                        kc = 2 * kc2 + j
                        nc.tensor.matmul(
                            t[:, j * 512 : (j + 1) * 512],
                            kTp_sb[:, h, kc * 128 : (kc + 1) * 128],
                            qkT_sb[:, hp, rc * 512 : (rc + 1) * 512],
                            start=True,
                            stop=True,
                        )
                    stp[h] = t
                stp_pend[g] = stp

            def emit_exp(g):
                hp, rc, kc2 = seq[g]
                stp = stp_pend.pop(g)
                ex2 = {}
                for h in (2 * hp, 2 * hp + 1):
                    ex = expool.tile([128, 1024], in_dt, tag="ex", name="ex")
                    nc.scalar.activation(
                        ex, stp[h], mybir.ActivationFunctionType.Exp, scale=SCALE
                    )
                    ex2[h] = ex
                ex_pend[g] = ex2

            def emit_pv(g):
                hp, rc, kc2 = seq[g]
                heads = (2 * hp, 2 * hp + 1)
                if kc2 == 0:
                    pv_cur[(hp, rc)] = {
                        h: pvhold.tile([128, 512], f32, tag="pvh", name="pvh")
                        for h in heads
                    }
                ex2 = ex_pend.pop(g)
                for h in heads:
                    for j in range(2):
                        kc = 2 * kc2 + j
                        nc.tensor.matmul(
                            pv_cur[(hp, rc)][h][: D + 1, :],
                            v_sb[:, kc, h, :],
                            ex2[h][:, j * 512 : (j + 1) * 512],
                            start=(kc == 0),
                            stop=(kc == 15),
                        )

            def emit_div(hp, rc):
                """Normalize: outT[0:64] = pv[0:64] / pv[64] per head.
                The custom-DVE fast reciprocal silently ignores input
                partition offsets, so first copy the denominator row from
                partition 64 down to a partition-0 tile."""
                heads = (2 * hp, 2 * hp + 1)
                pv = pv_cur.pop((hp, rc))
                dens, rbcs = {}, {}
                for h in heads:
                    dcp = dpool.tile([1, 512], f32, tag="dcp", name="dcp")
                    nc.vector.tensor_copy(dcp, pv[h][D : D + 1, :])
                    dens[h] = dpool.tile([1, 512], f32, tag="den", name="den")
                    nc.vector.reciprocal_approx_fast(dens[h], dcp)
                for h in heads:
                    rbcs[h] = dpool.tile([64, 512], f32, tag="rbc", name="rbc")
                    nc.gpsimd.partition_broadcast(rbcs[h], dens[h])
                for h in heads:
                    hb = (h % 2) * 64
                    nc.vector.tensor_tensor(
                        out=outT_sb[hb : hb + 64, hp, rc * 512 : (rc + 1) * 512],
                        in0=pv[h][0:D, :],
                        in1=rbcs[h][:],
                        op=mybir.AluOpType.mult,
                    )

            # fill work (qk/v/proj) interleaved into the attention steps'
            # PE slack.  g -> list of closures popped after that step's
            # lookahead STs and before its PVs.
            fill_plan = {g: [] for g in range(64)}
            fill_plan[0] = [
                lambda: v_chunk(0),
                lambda: v_chunk(1),
                lambda: qk_chunk(1, 1),
            ]
            fill_plan[1] = [
                lambda: v_chunk(2),
                lambda: v_chunk(3),
                lambda: qk_chunk(1, 2),
            ]
            fill_plan[2] = [
                lambda: v_chunk(4),
                lambda: v_chunk(5),
                lambda: qk_chunk(1, 3),
            ]
            fill_plan[3] = [
                lambda: v_chunk(6),
                lambda: v_chunk(7),
                lambda: qk_chunk(0, 1),
            ]
            fill_plan[4] = [lambda: v_chunk(8), lambda: v_chunk(9)]
            fill_plan[5] = [
                lambda: v_chunk(10),
                lambda: v_chunk(11),
                lambda: qk_chunk(2, 0),
            ]
            fill_plan[6] = [lambda: v_chunk(12), lambda: v_chunk(13)]
            fill_plan[7] = [lambda: v_chunk(14), lambda: v_chunk(15)]
            fill_plan[8] = [lambda: qk_chunk(3, 0)]
            fill_plan[9] = [lambda: qk_chunk(3, 1)]
            fill_plan[10] = [lambda: qk_chunk(3, 2)]
            fill_plan[11] = [lambda: qk_chunk(3, 3)]
            fill_plan[12] = [lambda: qk_chunk(0, 2)]
            fill_plan[14] = [lambda: qk_chunk(2, 1)]
            fill_plan[20] = [lambda: qk_chunk(0, 3)]
            fill_plan[22] = [lambda: qk_chunk(2, 2)]
            fill_plan[33] = [lambda: qk_chunk(2, 3)]
            # proj(rc) may only be emitted after BOTH (0,rc) and (1,rc)
            # divs are emitted (PE is in-order: an early proj MM waiting on
            # outT would deadlock behind itself).
            for i, g in enumerate(range(24, 32, 2)):  # proj(0) after (1,0)@g23
                fill_plan[g].append(lambda t=i: proj_pair(t))
            for i, g in enumerate(range(32, 40, 2)):  # proj(1) after (1,1)@g31
                fill_plan[g].append(lambda t=4 + i: proj_pair(t))
            for i, g in enumerate(range(48, 56, 2)):  # proj(2) after (1,2)@g47
                fill_plan[g].append(lambda t=8 + i: proj_pair(t))

            # prelude: minimum to start step g0 (q01 tokens 0:512 and k01
            # keys 0:512; v chunks follow as the first fills)
            qk_chunk(0, 0, big=True)
            qk_chunk(1, 0, big=True)

            emit_st(0)
            for g in range(64):
                hp, rc, kc2 = seq[g]
                emit_exp(g)
                if g + 1 < 64:
                    emit_st(g + 1)
                for fill in fill_plan[g]:
                    fill()
                emit_pv(g)
                if kc2 == 7:
                    emit_div(hp, rc)
                    if g == 63:
                        # keep the PE p-state high through the final div
                        # chain so the tail proj matmuls run at full clock
                        wps = pvps.tile([128, 512], f32, tag="pv", name="pv")
                        for _ in range(10):
                            nc.tensor.matmul(
                                wps, zq[:, :128], zq[:], start=True, stop=True
                            )

            # tail: proj(3) — output DMAs split across both HWDGE queues
            # (ScalarE is idle after the last exp)
            for t in range(12, 16):
                proj_pair(t, eng=nc.scalar if t % 2 else nc.sync)
    nc.compile()
    return nc


def make_in_maps(x, w_qkv, w_proj):
    in_maps = []
    for core in range(NCORES):
        b, g = core // 4, core % 4
        qs = slice(g * 256, (g + 1) * 256)
        q0 = g * 256
        in_maps.append(
            {
                "xT": _prep(x[b].T),
                # column order [q01 | k01 | q23 | k23] so the first-exp
                # critical DMA is just the first 256 columns
                "wqk": _prep(
                    np.concatenate(
                        [
                            w_qkv[:, q0 : q0 + 128],
                            w_qkv[:, C + q0 : C + q0 + 128],
                            w_qkv[:, q0 + 128 : q0 + 256],
                            w_qkv[:, C + q0 + 128 : C + q0 + 256],
                        ],
                        axis=1,
                    )
                ),
                "wv": _prep(w_qkv[:, 2 * C + g * 256 : 2 * C + (g + 1) * 256]),
                "wp": _prep(w_proj[qs, :]),
            }
        )
    return in_maps


def run_hw(x, w_qkv, w_proj, b_proj, trace=False):
    """Returns (full output [2, 2048, 1024] f32, exec_time_ns or None)."""
    in_maps = make_in_maps(x, w_qkv, w_proj)
    nc = build_nc()
    r = run_bass_kernel_spmd(nc, in_maps, core_ids=list(range(NCORES)), trace=trace)
    full = np.zeros((B, N, C), np.float32)
    for core in range(NCORES):
        full[core // 4] += np.asarray(r.results[core]["out"]).astype(np.float32)
    full += np.asarray(b_proj, np.float32)[None, None, :]
    return full, r.exec_time_ns


def kernel(**inputs):
    x = np.asarray(inputs["x"], np.float32)
    w_qkv = np.asarray(inputs["w_qkv"], np.float32)
    w_proj = np.asarray(inputs["w_proj"], np.float32)
    b_proj = np.asarray(inputs["b_proj"], np.float32)
    out, _ = run_hw(x, w_qkv, w_proj, b_proj, trace=False)
    return out


# revision 40
# speedup vs baseline: 1.2111x; 1.0090x over previous
"""Fused multi-head attention (B=2, N=2048, C=1024, H=16) on 8 TRN2 NeuronCores.

Sharding: core = (b, g) with b = batch (2) and g = head-group of 4 heads (4).
Each core computes, for its batch and 4 heads:
    qkv slice -> per-head softmax attention -> out-proj partial (row-parallel).
Host sums the 4 per-head-group proj partials per batch and adds b_proj.

Device algorithm (per core), matmuls in bf16:
  qkT/kT = (x @ Wqk)^T   [q/k feats on partitions, 2048 tokens]
  v      = x @ Wv        [2048 tokens, 4*64] (+ ones column per head)
  attention, software-pipelined over 64 global (block, kc2) steps:
    S^T tiles = matmul(lhsT=kTp_h (zero-padded K=128), rhs=q-chunk)
    expST = exp(S^T/8)  (ScalarE, PSUM->SBUF)
    outT[65, rows] += [v_h|1]^T-matmul expST  (row 64 = softmax denom)
  The ST matmuls for step g+1 are emitted BEFORE the PV matmuls of step g
  so the PE (strictly in-order) always has the next exp's input ready while
  ScalarE drains the current exps: steady state is ScalarE-bound at
  ~2.2us/step.  qk/v/proj fill work is interleaved into per-step slack.
  outT[0:64] *= 1/denominator  (fast DVE recip, GpSimd bcast, DVE mult)
  proj partials = out^T-matmul Wp -> DMA out
"""

import os

import numpy as np

import concourse.bass as bass
import concourse.mybir as mybir
import concourse.tile as tile
from concourse import bacc
from concourse.bass_utils import run_bass_kernel_spmd

B, N, C = 2, 2048, 1024
HC = 4  # heads per core
D = 64
NCORES = 8
KC = C // 128  # 8 contraction chunks for qkv
SCALE = D**-0.5  # 0.125

MM_DT = os.environ.get("ATTN_MM_DT", "bf16")


def _np_in_dtype():
    if MM_DT == "bf16":
        import ml_dtypes

        return np.dtype(ml_dtypes.bfloat16)
    return np.dtype(np.float32)


def _prep(a):
    """Cast to the device input dtype; for f32r, pre-round to TF32 (RTNE)."""
    a = np.ascontiguousarray(a)
    if MM_DT != "f32r":
        return a.astype(_np_in_dtype())
    u = a.astype(np.float32).view(np.uint32)
    u = (u + 0x0FFF + ((u >> 13) & 1)) & np.uint32(0xFFFFE000)
    return u.view(np.float32)


def build_nc():
    f32 = mybir.dt.float32
    in_dt = {
        "bf16": mybir.dt.bfloat16,
        "f32r": mybir.dt.float32r,
        "f32": mybir.dt.float32,
    }[MM_DT]

    nc = bacc.Bacc("TRN2", target_bir_lowering=False, debug=False, num_devices=NCORES)
    xT_d = nc.dram_tensor("xT", [C, N], in_dt, kind="ExternalInput").ap()
    wqk_d = nc.dram_tensor("wqk", [C, 2 * HC * D], in_dt, kind="ExternalInput").ap()
    wv_d = nc.dram_tensor("wv", [C, HC * D], in_dt, kind="ExternalInput").ap()
    wp_d = nc.dram_tensor("wp", [HC * D, C], in_dt, kind="ExternalInput").ap()
    # proj partials leave in bf16 (hosts sums in f32): halves out-DMA bytes;
    # adds ~1.7e-3 rel err (measured), well inside the margin
    out_d = nc.dram_tensor("out", [N, C], in_dt, kind="ExternalOutput").ap()

    with tile.TileContext(nc) as tc:
        with (
            tc.tile_pool(name="const", bufs=1) as const,
            tc.tile_pool(name="ex", bufs=8) as expool,
            tc.tile_pool(name="den", bufs=6) as dpool,
            tc.tile_pool(name="stage", bufs=4) as stage,
            tc.tile_pool(name="stps", bufs=2, space="PSUM") as stps,
            tc.tile_pool(name="pvps", bufs=2, space="PSUM") as pvps,
            tc.tile_pool(name="pvhold", bufs=2, space="PSUM") as pvhold,
        ):
            # persistent tiles
            # qkT/kT chunks: idx 0 = heads 0,1; idx 1 = heads 2,3
            #   (head even -> partitions 0:64, odd -> 64:128)
            qkT_sb = const.tile([128, 2, N], in_dt, tag="qkT")
            # kTp: per-head zero-padded K=128 stationary operand: head even
            #   has kT in rows 0:64 / zeros in 64:128, head odd the reverse,
            #   so full-array matmuls select one head's contraction.
            #   (64-row half-array ST matmuls measure wrong on HW when
            #   interleaved with full-array PV loads.)
            kTp_sb = const.tile([128, HC, N], in_dt, tag="kTp")
            v_sb = const.tile([128, 16, HC, D + 1], in_dt, tag="v")
            wp_sb = const.tile([128, 2, C], in_dt, tag="wp")
            outT_sb = const.tile([128, 2, N], in_dt, tag="outT")
            xT_sb = const.tile([128, KC, N], in_dt, tag="xT")
            wqk_sb = const.tile([128, KC, 2 * HC * D], in_dt, tag="wqk")
            wv_sb = const.tile([128, KC, HC * D], in_dt, tag="wv")

            # PE p-state warmers: dummy matmuls on a zeroed tile keep the PE
            # ramping to 2.4GHz during the DMA wait so the first real qk
            # psums don't run at the 0.65GHz cold clock
            zq = const.tile([128, 512], in_dt, tag="zq")
            nc.vector.memset(zq[:], 0.0)
            warm_ps = stps.tile([128, 1024], f32, tag="st", name="st")
            for i in range(12):
                nc.tensor.matmul(
                    warm_ps[:, :512], zq[:, :128], zq[:], start=True, stop=True
                )
            # warm the ScalarE Exp table during the DMA wait so the first
            # real activation doesn't pay the 1.3us table load
            wt_in = const.tile([1, 8], f32, tag="wtin")
            wt_out = const.tile([1, 8], f32, tag="wtout")
            nc.vector.memset(wt_in[:], 0.0)
            # zero the whole kTp on the (otherwise idle) GpSimd during the
            # DMA wait; the k copies then only fill their 64-row halves
            nc.gpsimd.memset(kTp_sb[:], 0.0)

            # ---- DMAs: batched issues split across the two HWDGE queues
            # (SP + ACT).  Host packs wqk columns as [q01|k01|q23|k23], so
            # the first-exp critical set is wqk[:, :256] + xT tokens 0:512.
            nc.sync.dma_start(
                wqk_sb[:, :, 0:256],
                wqk_d[:, 0:256].rearrange("(kc p) c -> p kc c", p=128),
            )
            nc.scalar.dma_start(
                xT_sb[:, :, 0:512],
                xT_d[:, 0:512].rearrange("(kc p) n -> p kc n", p=128),
            )
            nc.scalar.activation(
                wt_out, wt_in, mybir.ActivationFunctionType.Exp, scale=1.0
            )
            nc.sync.dma_start(wv_sb[:], wv_d.rearrange("(kc p) c -> p kc c", p=128))
            nc.sync.dma_start(
                wqk_sb[:, :, 256:512],
                wqk_d[:, 256:512].rearrange("(kc p) c -> p kc c", p=128),
            )
            for nt in range(1, 4):
                nc.scalar.dma_start(
                    xT_sb[:, :, nt * 512 : (nt + 1) * 512],
                    xT_d[:, nt * 512 : (nt + 1) * 512].rearrange(
                        "(kc p) n -> p kc n", p=128
                    ),
                )
            nc.sync.dma_start(wp_sb[:], wp_d.rearrange("(c2 p) c -> p c2 c", p=128))

            # ones column for the softmax-denominator trick
            ones_f32 = const.tile([128, 16, HC, 1], f32, tag="ones")
            nc.vector.memset(ones_f32[:], 1.0)
            nc.vector.tensor_copy(v_sb[:, :, :, D : D + 1], ones_f32[:])

            # ---- emission helpers ----
            def qk_chunk(mf, nt, big=False):
                """One psum of (x @ Wqk)^T: feat chunk mf, token chunk nt.
                wqk feat chunks (host order): 0 = q heads 0,1; 1 = k heads
                0,1; 2 = q heads 2,3; 3 = k heads 2,3."""
                if big:
                    ps = stps.tile([128, 1024], f32, tag="st", name="st")[:, :512]
                else:
                    ps = pvps.tile([128, 512], f32, tag="pv", name="pv")
                for kc in range(KC):
                    nc.tensor.matmul(
                        ps,
                        wqk_sb[:, kc, mf * 128 : (mf + 1) * 128],
                        xT_sb[:, kc, nt * 512 : (nt + 1) * 512],
                        start=(kc == 0),
                        stop=(kc == KC - 1),
                    )
                nts = slice(nt * 512, (nt + 1) * 512)
                if mf % 2 == 0:
                    nc.vector.tensor_copy(qkT_sb[:, mf // 2, nts], ps)
                else:
                    h0, h1 = 2 * (mf // 2), 2 * (mf // 2) + 1
                    nc.vector.tensor_copy(kTp_sb[0:64, h0, nts], ps[0:64, :])
                    nc.vector.tensor_copy(kTp_sb[64:128, h1, nts], ps[64:128, :])

            def v_chunk(t):
                """One psum of v = x @ Wv for token(=key) chunk t, all heads."""
                ps = pvps.tile([128, 512], f32, tag="pv", name="pv")[:, : HC * D]
                for kc in range(KC):
                    nc.tensor.matmul(
                        ps,
                        xT_sb[:, kc, t * 128 : (t + 1) * 128],
                        wv_sb[:, kc, :],
                        start=(kc == 0),
                        stop=(kc == KC - 1),
                    )
                nc.vector.tensor_copy(
                    v_sb[:, t, :, 0:D], ps.rearrange("p (h d) -> p h d", h=HC)
                )

            def proj_pair(t, eng=None):
                """partial[t*128:(t+1)*128, :] = out @ Wp, both column
                halves staged into one wide tile -> one 2KB-row DMA."""
                sg = stage.tile([128, 1024], in_dt, tag="sg", name="sg")
                for nf in range(2):
                    ps = pvps.tile([128, 512], f32, tag="pv", name="pv")
                    for c2 in range(2):
                        nc.tensor.matmul(
                            ps,
                            outT_sb[:, c2, t * 128 : (t + 1) * 128],
                            wp_sb[:, c2, nf * 512 : (nf + 1) * 512],
                            start=(c2 == 0),
                            stop=(c2 == 1),
                        )
                    nc.vector.tensor_copy(sg[:, nf * 512 : (nf + 1) * 512], ps)
                (eng or nc.sync).dma_start(out_d[t * 128 : (t + 1) * 128, :], sg)

            # ---- flat software-pipelined attention schedule ----
            # rc pairs complete early so only proj(3) remains in the tail
            blocks = [(0, 0), (0, 1), (1, 0), (1, 1), (0, 2), (1, 2), (0, 3), (1, 3)]
            seq = [(hp, rc, kc2) for hp, rc in blocks for kc2 in range(8)]

            stp_pend = {}
            ex_pend = {}
            pv_cur = {}

            def emit_st(g):
                hp, rc, kc2 = seq[g]
                stp = {}
                for h in (2 * hp, 2 * hp + 1):
                    t = stps.tile([128, 1024], f32, tag="st", name="st")
                    for j in range(2):
                        kc = 2 * kc2 + j
                        nc.tensor.matmul(
                            t[:, j * 512 : (j + 1) * 512],
                            kTp_sb[:, h, kc * 128 : (kc + 1) * 128],
                            qkT_sb[:, hp, rc * 512 : (rc + 1) * 512],
                            start=True,
                            stop=True,
                        )
                    stp[h] = t
                stp_pend[g] = stp

            def emit_exp(g):
                hp, rc, kc2 = seq[g]
                stp = stp_pend.pop(g)
                ex2 = {}
                for h in (2 * hp, 2 * hp + 1):
                    ex = expool.tile([128, 1024], in_dt, tag="ex", name="ex")
                    nc.scalar.activation(
                        ex, stp[h], mybir.ActivationFunctionType.Exp, scale=SCALE
                    )
                    ex2[h] = ex
                ex_pend[g] = ex2

            def emit_pv(g):
                hp, rc, kc2 = seq[g]
                heads = (2 * hp, 2 * hp + 1)
                if kc2 == 0:
                    pv_cur[(hp, rc)] = {
                        h: pvhold.tile([128, 512], f32, tag="pvh", name="pvh")
                        for h in heads
                    }
                ex2 = ex_pend.pop(g)
                for h in heads:
                    for j in range(2):
                        kc = 2 * kc2 + j
                        nc.tensor.matmul(
                            pv_cur[(hp, rc)][h][: D + 1, :],
                            v_sb[:, kc, h, :],
                            ex2[h][:, j * 512 : (j + 1) * 512],
                            start=(kc == 0),
                            stop=(kc == 15),
                        )

            def emit_div(hp, rc):
                """Normalize: outT[0:64] = pv[0:64] / pv[64] per head.
                The custom-DVE fast reciprocal silently ignores input
                partition offsets, so first copy the denominator row from
                partition 64 down to a partition-0 tile."""
                heads = (2 * hp, 2 * hp + 1)
                pv = pv_cur.pop((hp, rc))
                dens, rbcs = {}, {}
                for h in heads:
                    dcp = dpool.tile([1, 512], f32, tag="dcp", name="dcp")
                    nc.vector.tensor_copy(dcp, pv[h][D : D + 1, :])
                    dens[h] = dpool.tile([1, 512], f32, tag="den", name="den")
                    nc.vector.reciprocal_approx_fast(dens[h], dcp)
                for h in heads:
                    rbcs[h] = dpool.tile([64, 512], f32, tag="rbc", name="rbc")
                    nc.gpsimd.partition_broadcast(rbcs[h], dens[h])
                for h in heads:
                    hb = (h % 2) * 64
                    nc.vector.tensor_tensor(
                        out=outT_sb[hb : hb + 64, hp, rc * 512 : (rc + 1) * 512],
                        in0=pv[h][0:D, :],
                        in1=rbcs[h][:],
                        op=mybir.AluOpType.mult,
                    )

            # fill work (qk/v/proj) interleaved into the attention steps'
            # PE slack.  g -> list of closures popped after that step's
            # lookahead STs and before its PVs.
            fill_plan = {g: [] for g in range(64)}
            fill_plan[0] = [
                lambda: v_chunk(0),
                lambda: v_chunk(1),
                lambda: qk_chunk(1, 1),
            ]
            fill_plan[1] = [
                lambda: v_chunk(2),
                lambda: v_chunk(3),
                lambda: qk_chunk(1, 2),
            ]
            fill_plan[2] = [
                lambda: v_chunk(4),
                lambda: v_chunk(5),
                lambda: qk_chunk(1, 3),
            ]
            fill_plan[3] = [
                lambda: v_chunk(6),
                lambda: v_chunk(7),
                lambda: qk_chunk(0, 1),
            ]
            fill_plan[4] = [lambda: v_chunk(8), lambda: v_chunk(9)]
            fill_plan[5] = [lambda: v_chunk(10), lambda: v_chunk(11)]
            fill_plan[6] = [lambda: v_chunk(12), lambda: v_chunk(13)]
            fill_plan[7] = [lambda: v_chunk(14), lambda: v_chunk(15)]
            # later fills sit at mid-block steps (g%8 in 2..5): fills near a
            # boundary delay the next block's early STs and stall the exp
            # train there
            fill_plan[9] = [lambda: qk_chunk(2, 0)]
            fill_plan[10] = [lambda: qk_chunk(3, 0)]
            fill_plan[11] = [lambda: qk_chunk(3, 1)]
            fill_plan[12] = [lambda: qk_chunk(3, 2)]
            fill_plan[13] = [lambda: qk_chunk(3, 3)]
            fill_plan[18] = [lambda: qk_chunk(0, 2)]
            fill_plan[19] = [lambda: qk_chunk(2, 1)]
            fill_plan[30] = [lambda: qk_chunk(0, 3)]
            fill_plan[33] = [lambda: qk_chunk(2, 2)]
            fill_plan[42] = [lambda: qk_chunk(2, 3)]
            # proj(rc) may only be emitted after BOTH (0,rc) and (1,rc)
            # divs are emitted (PE is in-order: an early proj MM waiting on
            # outT would deadlock behind itself).
            for i, g in enumerate(range(26, 30)):  # proj(0) after (1,0)@g23
                fill_plan[g].append(lambda t=i: proj_pair(t))
            for i, g in enumerate(range(34, 38)):  # proj(1) after (1,1)@g31
                fill_plan[g].append(lambda t=4 + i: proj_pair(t))
            for i, g in enumerate(range(50, 54)):  # proj(2) after (1,2)@g47
                fill_plan[g].append(lambda t=8 + i: proj_pair(t))

            # prelude: minimum to start step g0 (q01 tokens 0:512 and k01
            # keys 0:512; v chunks follow as the first fills)
            qk_chunk(0, 0, big=True)
            qk_chunk(1, 0, big=True)

            emit_st(0)
            for g in range(64):
                hp, rc, kc2 = seq[g]
                emit_exp(g)
                if g + 1 < 64:
                    emit_st(g + 1)
                for fill in fill_plan[g]:
                    fill()
                emit_pv(g)
                if kc2 == 7:
                    emit_div(hp, rc)
                    if g == 63:
                        # keep the PE p-state high through the final div
                        # chain so the tail proj matmuls run at full clock
                        wps = pvps.tile([128, 512], f32, tag="pv", name="pv")
                        for _ in range(10):
                            nc.tensor.matmul(
                                wps, zq[:, :128], zq[:], start=True, stop=True
                            )

            # tail: proj(3) — output DMAs split across both HWDGE queues
            # (ScalarE is idle after the last exp)
            for t in range(12, 16):
                proj_pair(t, eng=nc.scalar if t % 2 else nc.sync)
    nc.compile()
    return nc


def make_in_maps(x, w_qkv, w_proj):
    in_maps = []
    for core in range(NCORES):
        b, g = core // 4, core % 4
        qs = slice(g * 256, (g + 1) * 256)
        q0 = g * 256
        in_maps.append(
            {
                "xT": _prep(x[b].T),
                # column order [q01 | k01 | q23 | k23] so the first-exp
                # critical DMA is just the first 256 columns
                "wqk": _prep(
                    np.concatenate(
                        [
                            w_qkv[:, q0 : q0 + 128],
                            w_qkv[:, C + q0 : C + q0 + 128],
                            w_qkv[:, q0 + 128 : q0 + 256],
                            w_qkv[:, C + q0 + 128 : C + q0 + 256],
                        ],
                        axis=1,
                    )
                ),
                "wv": _prep(w_qkv[:, 2 * C + g * 256 : 2 * C + (g + 1) * 256]),
                "wp": _prep(w_proj[qs, :]),
            }
        )
    return in_maps


def run_hw(x, w_qkv, w_proj, b_proj, trace=False):
    """Returns (full output [2, 2048, 1024] f32, exec_time_ns or None)."""
    in_maps = make_in_maps(x, w_qkv, w_proj)
    nc = build_nc()
    r = run_bass_kernel_spmd(nc, in_maps, core_ids=list(range(NCORES)), trace=trace)
    full = np.zeros((B, N, C), np.float32)
    for core in range(NCORES):
        full[core // 4] += np.asarray(r.results[core]["out"]).astype(np.float32)
    full += np.asarray(b_proj, np.float32)[None, None, :]
    return full, r.exec_time_ns


def kernel(**inputs):
    x = np.asarray(inputs["x"], np.float32)
    w_qkv = np.asarray(inputs["w_qkv"], np.float32)
    w_proj = np.asarray(inputs["w_proj"], np.float32)
    b_proj = np.asarray(inputs["b_proj"], np.float32)
    out, _ = run_hw(x, w_qkv, w_proj, b_proj, trace=False)
    return out
